# revision 1
# baseline (speedup 1.0000x reference)
"""Trainium2 Bass kernel for softmax-free attention:
    q = x @ Wq^T; k = x @ Wk^T; v = x @ Wv^T
    s = (q @ k^T) / sqrt(d); out = s @ v
  x: [4, 4096, 1024], W*: [1024, 1024], out: [4, 4096, 1024] (fp32)

Sharding: 8 cores; core c handles batch c//2, sequence-half c%2 (2048 query
rows). Each core projects q/k/v only for its OWN 2048 rows and spills k/v
into a cross-core-visible Shared-DRAM buffer (slot = own rank-in-pair via a
dynamic DMA offset). The pair partner reads both halves at local HBM
bandwidth — no bulk collective. Ordering across the pair is a tiny token
AllReduce (the token is DMA-sampled from the shared buffers, so it carries a
RAW dep on all spill writes); only the peer-slot reads wait on it, and they
start ~120us after it fires. The per-core x input is column-ROTATED on the
host (own half first); attention is permutation-invariant over m as long as
k and v use the same order.

Layout strategy: the PE contracts over the partition dim, so every operand is
arranged K-on-partitions via host-side pre-transposes (xT = x[b].T, W^T) and
chained matmuls that produce transposed outputs directly:
  qT[e,l] = sum_d WqT[d,e] xT[d,l]     (lhsT=WqT chunk, rhs=xT chunk)
  kT[e,m] = likewise
  v[m,d'] = sum_d xT[d,m] WvT[d,d']    (lhsT=xT chunk,  rhs=WvT chunk)
  sT[m,l] = sum_e kT[e,m] qT[e,l]      (lhsT=kT chunk,  rhs=qT chunk)
  out[l,d']= sum_m sT[m,l] v[m,d']     (lhsT=sT chunk,  rhs=v chunk)
The 1/sqrt(d) scale is folded into WqT on the host. All matmul inputs are
float32r (full PE rate at free-dim>=256, ~1e-4 rel err).

Phase A streams the own xT half once, producing kT + v (spilled to shared
DRAM) and qT (kept resident in SBUF). Phase B processes the 2048 query rows in two 1024-row
blocks, streaming kT/v back in 512-row m-chunks (4-matmul PSUM accumulation
groups keep the PE efficient) and accumulating out in SBUF via DVE adds.
"""

import sys
import types
from contextlib import ExitStack

import numpy as np

import concourse.bass as bass
import concourse.tile as tile
from concourse import bacc, mybir
from concourse.bass_utils import run_bass_kernel_spmd
from concourse.mybir import EngineType
from concourse.tile import add_dep_helper
from concourse.vector_clock import ScopedClock

# ---------------------------------------------------------------------------
# Environment shims
# ---------------------------------------------------------------------------


def _install_tile_drain_patch():
    """This toolchain's walrus caps sync waits at 1 per instruction, but
    TileContext's tail drain can carry several. Split the overflow onto
    preceding nops (same semantics: the issuing engine observes every sem
    before draining)."""
    if getattr(tile.TileContext, "_drain_patch_installed", False):
        return

    def _patched_drain_and_barrier(self, tick_clock, wait_clock):
        nc = self.nc
        collector = nc.sync.nop(hint="drain_wait_collector", nofuse=True)
        wait_clock.add_sem_waits(
            collector.ins, ScopedClock({None: tick_clock.global_clock})
        )
        waits = list(collector.ins.sync_info.on_wait or [])
        if len(waits) > 1:
            collector.ins.sync_info.on_wait = [waits[0]]
            for w in waits[1:]:
                nop = nc.sync.nop(hint="drain_wait_extra", nofuse=True)
                nop.ins.sync_info = mybir.SyncInfo(on_wait=[w], on_update=[])
        nc.sync.drain()

        nc.all_engine_barrier()
        assert self.sems is not None
        popped = nc._tile_sem_poison_stack.pop()
        assert popped is self._sem_poison
        nc.clear_and_free_semaphores(list(self.sems.allocated().values()))
        nc.all_engine_barrier()

    tile.TileContext._drain_and_barrier = _patched_drain_and_barrier
    tile.TileContext._drain_patch_installed = True


def _install_ntff_shim():
    """The image's antenv lacks axon_hooks, which silently degrades
    trace=True. Recreate the get/set pair and register the ctypes NTFF hook
    from trn_agent_boot (no-op if unavailable)."""
    if "antenv.axon_hooks" in sys.modules:
        return
    state = {"hook": None}

    def set_axon_ntff_profile_hook(h):
        state["hook"] = h

    def get_axon_ntff_profile_hook():
        return state["hook"]

    mod = types.ModuleType("antenv.axon_hooks")
    mod.set_axon_ntff_profile_hook = set_axon_ntff_profile_hook
    mod.get_axon_ntff_profile_hook = get_axon_ntff_profile_hook
    sys.modules["antenv.axon_hooks"] = mod
    try:
        import antenv

        antenv.axon_hooks = mod
        from trn_agent_boot.trn_boot import _ntff_profile_via_ctypes

        set_axon_ntff_profile_hook(
            _ntff_profile_via_ctypes("/opt/axon/libaxon_pjrt.so")
        )
    except Exception:
        pass


_install_tile_drain_patch()
_install_ntff_shim()

# ---------------------------------------------------------------------------
# Problem constants (hardcoded per the harness contract)
# ---------------------------------------------------------------------------

B, L, D = 4, 4096, 1024
N_CORES = 8
P = 128
LH = L // 2  # query rows per core
DC = D // P  # 8 contraction chunks of 128 over d/e
F32 = mybir.dt.float32
F32R = mybir.dt.float32r

ACHUNK = 512  # phase-A xT column chunk
BCHUNK = 512  # phase-B m-chunk (k/v rows)
PAIRS = [[2 * i, 2 * i + 1] for i in range(N_CORES // 2)]
BMS = BCHUNK // P
LBLOCK = 1024  # phase-B query-block rows


def build_nc():
    nc = bacc.Bacc("TRN2", target_bir_lowering=False, debug=False,
                   num_devices=N_CORES)
    xTr = nc.dram_tensor("xTr", [D, L], F32, kind="ExternalInput").ap()
    wqT = nc.dram_tensor("wqT", [D, D], F32, kind="ExternalInput").ap()
    wkT = nc.dram_tensor("wkT", [D, D], F32, kind="ExternalInput").ap()
    wvT = nc.dram_tensor("wvT", [D, D], F32, kind="ExternalInput").ap()
    out = nc.dram_tensor("out", [LH, D], F32, kind="ExternalOutput").ap()
    slots = nc.dram_tensor("slots", [1, 2], mybir.dt.uint32,
                           kind="ExternalInput").ap()
    kTsh = nc.dram_tensor("kTsh", [2, D, LH], F32R, addr_space="Shared").ap()
    vsh = nc.dram_tensor("vsh", [2, LH, D], F32R, addr_space="Shared").ap()
    tok = nc.dram_tensor("tok", [1, 2], F32).ap()
    tok2 = nc.dram_tensor("tok2", [1, 2], F32).ap()
    wu_sink = nc.dram_tensor("wu_sink", [P, ACHUNK], F32).ap()

    def chunked(ap):  # [K*, N] dram -> [P, K*/P, N] partition-major
        return ap.rearrange("(c p) n -> p c n", p=P)

    with tile.TileContext(nc) as tc, ExitStack() as octx:
        psum = octx.enter_context(tc.tile_pool(name="psum", bufs=8, space="PSUM"))
        qpool = octx.enter_context(tc.tile_pool(name="qpool", bufs=1))
        qsb = qpool.tile([P, DC, LH], F32R, tag="qsb")  # qT, SBUF-resident

        # HAM warmup: ~17us of junk matmuls while the first DMAs load, so the
        # PE clock gate is already at 8/8 when real work arrives
        with tc.tile_pool(name="wupool", bufs=1) as wupool:
            wut = wupool.tile([P, ACHUNK], F32R, tag="wut")
            nc.vector.memset(wut[:].bitcast(F32), 0.0)
            wuo = wupool.tile([P, ACHUNK], F32, tag="wuo")
            for g in range(20):
                wp = psum.tile([P, ACHUNK], F32, tag="ps", name=f"wu_{g}")
                for r in range(2):
                    nc.tensor.matmul(wp[:], wut[:, 0:P], wut[:],
                                     start=(r == 0), stop=(r == 1))
                if g == 19:
                    nc.vector.tensor_copy(wuo[:], wp[:])
            nc.sync.dma_start(wu_sink[:], wuo[:])

        # ---------------- Phase A: projections from one xT stream ----------
        # one interleaved loop (k, v, q per chunk — no intra-A transitions);
        # spills batched at 512KB so the sync ring's issue overhead fits the
        # per-chunk compute budget
        with ExitStack() as actx:
            wpool = actx.enter_context(tc.tile_pool(name="wpool", bufs=1))
            xpool = actx.enter_context(tc.tile_pool(name="xpool", bufs=2))
            stage = actx.enter_context(tc.tile_pool(name="stage", bufs=2))

            wq = wpool.tile([P, DC, D], F32R, tag="wq")
            wk = wpool.tile([P, DC, D], F32R, tag="wk")
            wv = wpool.tile([P, DC, D], F32R, tag="wv")
            # spread initial loads over rings: first MMs need wk + xc0 only,
            # and only their first c-chunks — split wk per chunk
            for c in range(DC):
                nc.sync.dma_start(wk[:, c], chunked(wkT).bitcast(F32R)[:, c])
            nc.gpsimd.dma_start(wv[:], chunked(wvT).bitcast(F32R))
            nc.gpsimd.dma_start(wq[:], chunked(wqT).bitcast(F32R))

            # rank-in-pair slot selectors for the shared spill buffers
            st_sl = stage.tile([1, 2], mybir.dt.uint32, tag="sl", bufs=1)
            nc.sync.dma_start(st_sl[:], slots[:])
            regs_o = nc.alloc_registers(
                engines=[EngineType.SP, EngineType.Activation])
            nc.regs_load(regs_o, st_sl[0:1, 0:1])
            svo = nc.snap(regs_o, donate=True)
            regs_p = nc.alloc_registers(
                engines=[EngineType.SP, EngineType.Activation])
            nc.regs_load(regs_p, st_sl[0:1, 1:2])
            svp = nc.snap(regs_p, donate=True)

            for j in range(LH // ACHUNK):
                xc = xpool.tile([P, DC, ACHUNK], F32R, tag="xc")
                cols = slice(j * ACHUNK, (j + 1) * ACHUNK)
                if j == 0:
                    # split per c-chunk: the first accumulation group starts
                    # after 768KB instead of 6MB of DMA
                    for c in range(DC):
                        nc.scalar.dma_start(
                            xc[:, c], chunked(xTr[:, cols]).bitcast(F32R)[:, c])
                else:
                    nc.scalar.dma_start(xc[:], chunked(xTr[:, cols]).bitcast(F32R))

                # kT chunk -> spill (four 512KB batches of 2 e-tiles)
                for eh in range(4):
                    kst = stage.tile([P, 2, ACHUNK], F32R, tag="st")
                    for ei in range(2):
                        e = eh * 2 + ei
                        pt = psum.tile([P, ACHUNK], F32, tag="ps")
                        for c in range(DC):
                            nc.tensor.matmul(
                                pt[:], wk[:, c, e * P:(e + 1) * P], xc[:, c],
                                start=(c == 0), stop=(c == DC - 1))
                        nc.vector.tensor_copy(kst[:, ei], pt[:])
                    nc.sync.dma_start(
                        kTsh[bass.ds(svo, 1), eh * 2 * P:(eh + 1) * 2 * P,
                             cols].rearrange("s (c p) n -> p (s c) n", p=P),
                        kst[:])

                # v chunk -> spill (four 512KB batches of 1 row-tile)
                for ms in range(ACHUNK // P):
                    row0 = j * ACHUNK + ms * P
                    vst = stage.tile([P, 1, D], F32R, tag="st", name=f"vst_{j}_{ms}")
                    for dh in range(D // ACHUNK):
                        pt = psum.tile([P, ACHUNK], F32, tag="ps")
                        dsl = slice(dh * ACHUNK, (dh + 1) * ACHUNK)
                        for c in range(DC):
                            nc.tensor.matmul(
                                pt[:], xc[:, c, ms * P:(ms + 1) * P],
                                wv[:, c, dsl],
                                start=(c == 0), stop=(c == DC - 1))
                        nc.vector.tensor_copy(vst[:, 0, dsl], pt[:])
                    nc.sync.dma_start(
                        vsh[bass.ds(svo, 1), row0:row0 + P, :].rearrange(
                            "s (c p) n -> p (s c) n", p=P),
                        vst[:])

                # qT chunk (first half of the rotated stream) -> resident
                if j < LH // ACHUNK:
                    for e in range(DC):
                        pt = psum.tile([P, ACHUNK], F32, tag="ps")
                        for c in range(DC):
                            nc.tensor.matmul(
                                pt[:], wq[:, c, e * P:(e + 1) * P], xc[:, c],
                                start=(c == 0), stop=(c == DC - 1))
                        nc.vector.tensor_copy(qsb[:, e, cols], pt[:])

            # pair barrier: the token is sampled from the shared buffers, so
            # its DMA carries a RAW dep on every spill write; the AllReduce
            # completes only when BOTH pair members' spills are durable
            tkt = stage.tile([1, 2], F32, tag="tkt", bufs=1)
            nc.sync.dma_start(tkt[0:1, 0:1], kTsh[bass.ds(svo, 1), 0:1, 0:1]
                              .rearrange("s c n -> c s n").bitcast(F32))
            nc.sync.dma_start(tkt[0:1, 1:2], vsh[bass.ds(svo, 1), 0:1, 0:1]
                              .rearrange("s c n -> c s n").bitcast(F32))
            nc.sync.dma_start(tok[:], tkt[:])
            pair_barrier = nc.gpsimd.collective_compute(
                "AllReduce", mybir.AluOpType.add, replica_groups=PAIRS,
                ins=[tok], outs=[tok2])

        # ---------------- Phase B: attention over m, single query block ----
        with ExitStack() as bctx:
            opool = bctx.enter_context(tc.tile_pool(name="opool", bufs=1))
            kpool = bctx.enter_context(tc.tile_pool(name="kpool", bufs=2))
            vpool = bctx.enter_context(tc.tile_pool(name="vpool", bufs=2))
            spool = bctx.enter_context(tc.tile_pool(name="spool", bufs=2))

            for lb in range(LH // LBLOCK):
                lsl0 = lb * LBLOCK
                ob = opool.tile([P, LBLOCK // P, D], F32, tag="ob")

                for j in range(L // BCHUNK):
                    # chunks 0-3: own half; 4-7: peer half (after the barrier)
                    own = j < LH // BCHUNK
                    sl = svo if own else svp
                    jj = j % (LH // BCHUNK)
                    msl = slice(jj * BCHUNK, (jj + 1) * BCHUNK)
                    kc = kpool.tile([P, DC, BCHUNK], F32R, tag="kc")
                    kld = nc.sync.dma_start(kc[:], kTsh[
                        bass.ds(sl, 1), :, msl].rearrange(
                        "s (c p) m -> p (s c) m", p=P))
                    vc = vpool.tile([P, BMS, D], F32R, tag="vc")
                    vld = nc.scalar.dma_start(vc[:], vsh[
                        bass.ds(sl, 1), msl, :].rearrange(
                        "s (c p) n -> p (s c) n", p=P))
                    if not own:
                        add_dep_helper(kld.ins, pair_barrier.ins,
                                       reason="peer kc after pair barrier")
                        add_dep_helper(vld.ins, pair_barrier.ins,
                                       reason="peer vc after pair barrier")

                    # sT chunk: [BCHUNK(m), LBLOCK(l)] as BMS tiles [P, LBLOCK]
                    sc = spool.tile([P, BMS, LBLOCK], F32R, tag="sc")
                    for ms in range(BMS):
                        for lh in range(LBLOCK // ACHUNK):
                            pt = psum.tile([P, ACHUNK], F32, tag="ps")
                            ls = slice(lh * ACHUNK, (lh + 1) * ACHUNK)
                            for e in range(DC):
                                nc.tensor.matmul(
                                    pt[:], kc[:, e, ms * P:(ms + 1) * P],
                                    qsb[:, e, lsl0 + lh * ACHUNK:
                                        lsl0 + (lh + 1) * ACHUNK],
                                    start=(e == 0), stop=(e == DC - 1))
                            nc.vector.tensor_copy(sc[:, ms, ls], pt[:])

                    # out += sT^T @ v, accumulated into ob
                    for lt in range(LBLOCK // P):
                        for dh in range(D // ACHUNK):
                            pt = psum.tile([P, ACHUNK], F32, tag="ps")
                            dsl = slice(dh * ACHUNK, (dh + 1) * ACHUNK)
                            for ms in range(BMS):
                                nc.tensor.matmul(
                                    pt[:], sc[:, ms, lt * P:(lt + 1) * P],
                                    vc[:, ms, dsl],
                                    start=(ms == 0), stop=(ms == BMS - 1))
                            if j == 0:
                                nc.vector.tensor_copy(ob[:, lt, dsl], pt[:])
                            else:
                                nc.vector.tensor_add(
                                    ob[:, lt, dsl], ob[:, lt, dsl], pt[:])

                # per-tile stores so the tail overlaps the last flush-adds
                for lt in range(LBLOCK // P):
                    row0 = lsl0 + lt * P
                    nc.sync.dma_start(out[row0:row0 + P, :], ob[:, lt])

    nc.compile()
    return nc


_NC_CACHE = {}


def _get_nc():
    if "nc" not in _NC_CACHE:
        _NC_CACHE["nc"] = build_nc()
    return _NC_CACHE["nc"]


def run(inputs, trace=False):
    """Run the kernel on all 8 cores. Returns (full_output, BassKernelResults)."""
    x = np.asarray(inputs["x"], dtype=np.float32)
    Wq = np.asarray(inputs["Wq"], dtype=np.float32)
    Wk = np.asarray(inputs["Wk"], dtype=np.float32)
    Wv = np.asarray(inputs["Wv"], dtype=np.float32)

    xT = np.ascontiguousarray(x.transpose(0, 2, 1))  # [B, D, L]
    inv_sqrt_d = np.float32(1.0 / np.sqrt(D))
    wqT = np.ascontiguousarray(Wq.T * inv_sqrt_d)
    wkT = np.ascontiguousarray(Wk.T)
    wvT = np.ascontiguousarray(Wv.T)

    in_maps = []
    for c in range(N_CORES):
        b, h = c // 2, c % 2
        # rotate columns so this core's own half comes first
        xtb = xT[b]
        xtr = np.concatenate(
            [xtb[:, h * LH:(h + 1) * LH], xtb[:, (1 - h) * LH:(2 - h) * LH]],
            axis=1)
        in_maps.append({
            "xTr": np.ascontiguousarray(xtr),
            "slots": np.array([[h, 1 - h]], dtype=np.uint32),
            "wqT": wqT, "wkT": wkT, "wvT": wvT,
        })

    nc = _get_nc()
    res = run_bass_kernel_spmd(nc, in_maps, list(range(N_CORES)), trace=trace)

    full = np.empty((B, L, D), dtype=np.float32)
    for c in range(N_CORES):
        b, h = c // 2, c % 2
        full[b, h * LH:(h + 1) * LH, :] = res.results[c]["out"]
    return full, res


def kernel(**inputs):
    full, _ = run(inputs, trace=False)
    return full



# revision 5
# speedup vs baseline: 2.7140x; 2.7140x over previous
"""Trainium2 Bass kernel for softmax-free attention:
    q = x @ Wq^T; k = x @ Wk^T; v = x @ Wv^T
    s = (q @ k^T) / sqrt(d); out = s @ v
  x: [4, 4096, 1024], W*: [1024, 1024], out: [4, 4096, 1024] (fp32)

Softmax-free attention is LINEAR, so matmul associativity applies:
    out_b = x_b Wq^T Wk x_b^T x_b Wv^T / sqrt(d)
          = x_b @ M_b,   M_b = Wq_s^T Wk C_b Wv^T,   C_b = x_b^T x_b
with Wq_s = Wq / sqrt(d). This cuts total MACs from 206e9 (explicit
[L,L] score matrix) to ~44e9: per core 256 MMs for C, 256 for the
M-chain, 256 for x@M — 768 N=512 matmuls vs 3600+ in the naive form.

Sharding: 8 cores; core c handles batch c//2, half h = c%2.
  Phase A: core streams its OWN 2048 rows of x_b (natural layout) and
    computes C_own = x_own^T x_own (full [D,D], contraction over its
    rows). C_b = C_own + C_peer. C_own is spilled to cross-core-visible
    Shared DRAM in four column-quarter batches, each followed by its own
    tiny token AllReduce over the pair, so the peer's quarters become
    readable in a pipeline instead of one late barrier.
  Phase B1: U = (C_own + C_peer) @ WvT[:, own 512 cols] accumulated in
    single PSUM groups (own-part MMs run while peer quarters arrive);
    V = Wk @ U; Mc = Wq_s^T @ V = M[:, own cols]. Mc is exchanged with
    the pair partner (spill + token AllReduce) while phase B2 starts on
    the own half. C/U/V/Mc rows are global d indices on all cores; only
    the 512-column j-slice is core-specific (via the pre-sliced WvT
    input), so the SPMD program is identical across cores.
  Phase B2: out[own 2048 rows, :] = x_own @ [Mc_own | Mc_peer], with the
    own column half computed first to hide the Mc exchange. The output
    dram is [2048, 2, 512] with a dynamic slot offset so the local
    own/peer column order maps back to global column halves.

Layout strategy: the PE contracts over the partition dim. C is computed
from x in NATURAL layout (rows on partitions); all later stages need
d-on-partitions operands, which fall out of the previous stage's PSUM
orientation or host-side pre-transposes (xT, Wk^T). C is symmetric, so
C row-chunks serve directly as lhsT tiles without any transpose. The
1/sqrt(d) scale is folded into Wq on the host. All matmul inputs are
float32r (full PE rate at free-dim>=256, ~1e-4 rel err).
"""

import sys
import types
from contextlib import ExitStack

import numpy as np

import concourse.bass as bass
import concourse.tile as tile
from concourse import bacc, mybir
from concourse.bass_utils import run_bass_kernel_spmd
from concourse.mybir import EngineType
from concourse.tile import add_dep_helper
from concourse.vector_clock import ScopedClock

# ---------------------------------------------------------------------------
# Environment shims
# ---------------------------------------------------------------------------


def _install_tile_drain_patch():
    """This toolchain's walrus caps sync waits at 1 per instruction, but
    TileContext's tail drain can carry several. Split the overflow onto
    preceding nops (same semantics: the issuing engine observes every sem
    before draining)."""
    if getattr(tile.TileContext, "_drain_patch_installed", False):
        return

    def _patched_drain_and_barrier(self, tick_clock, wait_clock):
        nc = self.nc
        collector = nc.sync.nop(hint="drain_wait_collector", nofuse=True)
        wait_clock.add_sem_waits(
            collector.ins, ScopedClock({None: tick_clock.global_clock})
        )
        waits = list(collector.ins.sync_info.on_wait or [])
        if len(waits) > 1:
            collector.ins.sync_info.on_wait = [waits[0]]
            for w in waits[1:]:
                nop = nc.sync.nop(hint="drain_wait_extra", nofuse=True)
                nop.ins.sync_info = mybir.SyncInfo(on_wait=[w], on_update=[])
        nc.sync.drain()

        nc.all_engine_barrier()
        assert self.sems is not None
        popped = nc._tile_sem_poison_stack.pop()
        assert popped is self._sem_poison
        nc.clear_and_free_semaphores(list(self.sems.allocated().values()))
        nc.all_engine_barrier()

    tile.TileContext._drain_and_barrier = _patched_drain_and_barrier
    tile.TileContext._drain_patch_installed = True


def _install_ntff_shim():
    """The image's antenv lacks axon_hooks, which silently degrades
    trace=True. Recreate the get/set pair and register the ctypes NTFF hook
    from trn_agent_boot (no-op if unavailable)."""
    if "antenv.axon_hooks" in sys.modules:
        return
    state = {"hook": None}

    def set_axon_ntff_profile_hook(h):
        state["hook"] = h

    def get_axon_ntff_profile_hook():
        return state["hook"]

    mod = types.ModuleType("antenv.axon_hooks")
    mod.set_axon_ntff_profile_hook = set_axon_ntff_profile_hook
    mod.get_axon_ntff_profile_hook = get_axon_ntff_profile_hook
    sys.modules["antenv.axon_hooks"] = mod
    try:
        import antenv

        antenv.axon_hooks = mod
        from trn_agent_boot.trn_boot import _ntff_profile_via_ctypes

        set_axon_ntff_profile_hook(
            _ntff_profile_via_ctypes("/opt/axon/libaxon_pjrt.so")
        )
    except Exception:
        pass


_install_tile_drain_patch()
_install_ntff_shim()

# ---------------------------------------------------------------------------
# Problem constants (hardcoded per the harness contract)
# ---------------------------------------------------------------------------

B, L, D = 4, 4096, 1024
N_CORES = 8
P = 128
LH = L // 2  # rows per core
HD = D // 2  # own output-column half
DC = D // P  # 8 contraction chunks of 128 over d/e
MC = LH // P  # 16 m-chunks of own rows
F32 = mybir.dt.float32
F32R = mybir.dt.float32r

PAIRS = [[2 * i, 2 * i + 1] for i in range(N_CORES // 2)]
QCOL = D // 4  # C spill column-quarter width (256)


def build_nc():
    nc = bacc.Bacc("TRN2", target_bir_lowering=False, debug=False,
                   num_devices=N_CORES)
    xn = nc.dram_tensor("xn", [LH, D], F32, kind="ExternalInput").ap()
    xth = nc.dram_tensor("xth", [D, LH], F32, kind="ExternalInput").ap()
    wq = nc.dram_tensor("wq", [D, D], F32, kind="ExternalInput").ap()
    wkT = nc.dram_tensor("wkT", [D, D], F32, kind="ExternalInput").ap()
    wvT = nc.dram_tensor("wvT", [D, HD], F32, kind="ExternalInput").ap()
    out = nc.dram_tensor("out", [2, LH, HD], F32, kind="ExternalOutput").ap()
    slots = nc.dram_tensor("slots", [1, 2], mybir.dt.uint32,
                           kind="ExternalInput").ap()
    Csh = nc.dram_tensor("Csh", [2, D, D], F32R, addr_space="Shared").ap()
    Msh = nc.dram_tensor("Msh", [2, D, HD], F32R, addr_space="Shared").ap()
    tokc = [nc.dram_tensor(f"tokc{q}", [1, 2], F32).ap() for q in range(4)]
    tokco = [nc.dram_tensor(f"tokc{q}o", [1, 2], F32).ap() for q in range(4)]
    tokm = nc.dram_tensor("tokm", [1, 2], F32).ap()
    tokmo = nc.dram_tensor("tokmo", [1, 2], F32).ap()
    wu_sink = nc.dram_tensor("wu_sink", [P, 512], F32).ap()

    def chunked(ap):  # [K*, N] dram -> [P, K*/P, N] partition-major
        return ap.rearrange("(c p) n -> p c n", p=P)

    with tile.TileContext(nc) as tc, ExitStack() as octx:
        psum = octx.enter_context(tc.tile_pool(name="psum", bufs=8, space="PSUM"))

        # ---- long-lived left-stack pools -------------------------------
        stage = octx.enter_context(tc.tile_pool(name="stage", bufs=2))
        wkpool = octx.enter_context(tc.tile_pool(name="wkpool", bufs=1))
        wksb = wkpool.tile([P, DC, D], F32R, tag="wk")  # Wk^T, resident

        # rank-in-pair slot selectors for the shared spill buffers
        st_sl = stage.tile([1, 2], mybir.dt.uint32, tag="sl", bufs=1)
        nc.sync.dma_start(st_sl[:], slots[:])
        regs_o = nc.alloc_registers(
            engines=[EngineType.SP, EngineType.Activation])
        nc.regs_load(regs_o, st_sl[0:1, 0:1])
        svo = nc.snap(regs_o, donate=True)
        regs_p = nc.alloc_registers(
            engines=[EngineType.SP, EngineType.Activation])
        nc.regs_load(regs_p, st_sl[0:1, 1:2])
        svp = nc.snap(regs_p, donate=True)

        # HAM warmup: junk matmuls while the first DMAs load, so the PE
        # clock gate is at 8/8 when real work arrives
        with tc.tile_pool(name="wupool", bufs=1) as wupool:
            wut = wupool.tile([P, 512], F32R, tag="wut")
            nc.vector.memset(wut[:].bitcast(F32), 0.0)
            wuo = wupool.tile([P, 512], F32, tag="wuo")
            for g in range(8):
                wp = psum.tile([P, 512], F32, tag="ps", name=f"wu_{g}")
                for r in range(2):
                    nc.tensor.matmul(wp[:], wut[:, 0:P], wut[:],
                                     start=(r == 0), stop=(r == 1))
                if g == 7:
                    nc.vector.tensor_copy(wuo[:], wp[:])
            nc.sync.dma_start(wu_sink[:], wuo[:])

        with ExitStack() as cctx:
            cpool = cctx.enter_context(tc.tile_pool(name="cpool", bufs=1))
            csb = cpool.tile([P, DC, D], F32R, tag="csb")  # C_own rows chunked
            wvpool = cctx.enter_context(tc.tile_pool(name="wvpool", bufs=1))
            wvsb = wvpool.tile([P, DC, HD], F32R, tag="wv")
            cppool = cctx.enter_context(tc.tile_pool(name="cppool", bufs=2))

            # weight prefetches on the gpsimd queue
            nc.gpsimd.dma_start(wvsb[:], chunked(wvT).bitcast(F32R))
            nc.gpsimd.dma_start(wksb[:], chunked(wkT).bitcast(F32R))

            # ------------- Phase A: C_own = xn^T xn -----------------------
            with ExitStack() as actx:
                xpool = actx.enter_context(tc.tile_pool(name="xpool", bufs=1))
                xnsb = xpool.tile([P, MC, D], F32R, tag="xn")
                # stream own rows, alternating two DMA queues
                for m in range(MC):
                    eng = nc.scalar if (m % 2 == 0) else nc.sync
                    eng.dma_start(xnsb[:, m], chunked(xn).bitcast(F32R)[:, m])

                for d2h in range(2):
                    pts = []
                    for d1 in range(DC):
                        pt = psum.tile([P, 512], F32, tag="ps",
                                       name=f"c_{d2h}_{d1}")
                        pts.append(pt)
                    for m in range(MC):
                        for d1 in range(DC):
                            nc.tensor.matmul(
                                pts[d1][:],
                                xnsb[:, m, d1 * P:(d1 + 1) * P],
                                xnsb[:, m, d2h * 512:(d2h + 1) * 512],
                                start=(m == 0), stop=(m == MC - 1))
                    for d1 in range(DC):
                        nc.vector.tensor_copy(
                            csb[:, d1, d2h * 512:(d2h + 1) * 512], pts[d1][:])
                    # spill this half's two column-quarters + their tokens
                    for qq in range(2):
                        q = d2h * 2 + qq
                        cs = slice(q * QCOL, (q + 1) * QCOL)
                        nc.sync.dma_start(
                            Csh[bass.ds(svo, 1), :, cs].rearrange(
                                "s (c p) n -> p (s c) n", p=P),
                            csb[:, :, cs])
                        tkt = stage.tile([1, 2], F32, tag="tkt",
                                         name=f"tktc{q}")
                        for e in range(2):
                            nc.sync.dma_start(
                                tkt[0:1, e:e + 1],
                                Csh[bass.ds(svo, 1), e:e + 1,
                                    q * QCOL:q * QCOL + 1].rearrange(
                                    "s c n -> c s n").bitcast(F32))
                        nc.sync.dma_start(tokc[q][:], tkt[:])
                # pair barriers: quarter q's AllReduce completes only when
                # BOTH pair members' quarter-q spills are durable
                cbars = []
                for q in range(4):
                    bar = nc.gpsimd.collective_compute(
                        "AllReduce", mybir.AluOpType.add,
                        replica_groups=PAIRS, ins=[tokc[q]], outs=[tokco[q]])
                    cbars.append(bar)

            # ------------- Phase B1: U = C @ WvT_own ----------------------
            # right-stack pools for the B-phase long-lived tensors
            bpool = tc.alloc_tile_pool(name="bpool", bufs=1, side="right")
            octx.callback(bpool.release)
            wqsb = bpool.tile([P, DC, D], F32R, tag="wq")
            usb = bpool.tile([P, DC, HD], F32R, tag="usb")
            vsb = bpool.tile([P, DC, HD], F32R, tag="vsb")
            mcsb = bpool.tile([P, DC, D], F32R, tag="mcsb")
            nc.gpsimd.dma_start(wqsb[:], chunked(wq).bitcast(F32R))

            upts = []
            for d1 in range(DC):
                pt = psum.tile([P, 512], F32, tag="ps", name=f"u_{d1}")
                upts.append(pt)
            # own-part MMs: run immediately, covering the barrier latency
            for d1 in range(DC):
                for c in range(DC):
                    nc.tensor.matmul(
                        upts[d1][:], csb[:, c, d1 * P:(d1 + 1) * P],
                        wvsb[:, c], start=(c == 0), stop=False)
            # peer-part MMs: column-quarter q of C_peer serves U groups
            # {2q, 2q+1}; quarters stream in as their barriers complete
            for q in range(4):
                cp = cppool.tile([P, DC, QCOL], F32R, tag="cp",
                                 name=f"cp_{q}")
                ld = nc.scalar.dma_start(
                    cp[:], Csh[bass.ds(svp, 1), :,
                               q * QCOL:(q + 1) * QCOL].rearrange(
                        "s (c p) n -> p (s c) n", p=P))
                add_dep_helper(ld.ins, cbars[q].ins,
                               reason="peer C quarter after pair barrier")
                for dq in range(2):
                    d1 = q * 2 + dq
                    for c in range(DC):
                        nc.tensor.matmul(
                            upts[d1][:], cp[:, c, dq * P:(dq + 1) * P],
                            wvsb[:, c], start=False, stop=(c == DC - 1))
                for dq in range(2):
                    d1 = q * 2 + dq
                    nc.vector.tensor_copy(usb[:, d1], upts[d1][:])

        # csb/wvsb/cpeer released here; xth takes the space
        xthpool = tc.alloc_tile_pool(name="xthpool", bufs=1, side="right")
        octx.callback(xthpool.release)
        xthsb = xthpool.tile([P, DC, LH], F32R, tag="xth")
        for g in range(4):
            nc.scalar.dma_start(
                xthsb[:, :, g * 512:(g + 1) * 512],
                chunked(xth).bitcast(F32R)[:, :, g * 512:(g + 1) * 512])
        opool = tc.alloc_tile_pool(name="opool", bufs=2, side="right")
        octx.callback(opool.release)

        # ------------- V = Wk @ U, chunk-outer so V pipelines behind U ----
        vpts = []
        for eb in range(DC):
            vpts.append(psum.tile([P, 512], F32, tag="ps", name=f"v_{eb}"))
        for c in range(DC):
            for eb in range(DC):
                nc.tensor.matmul(
                    vpts[eb][:], wksb[:, c, eb * P:(eb + 1) * P],
                    usb[:, c], start=(c == 0), stop=(c == DC - 1))
        for eb in range(DC):
            nc.vector.tensor_copy(vsb[:, eb], vpts[eb][:])

        # ------------- Mc = Wq_s^T @ V = M[:, own cols] -------------------
        mpts = []
        for ab in range(DC):
            mpts.append(psum.tile([P, 512], F32, tag="ps", name=f"m_{ab}"))
        for c in range(DC):
            for ab in range(DC):
                nc.tensor.matmul(
                    mpts[ab][:], wqsb[:, c, ab * P:(ab + 1) * P],
                    vsb[:, c], start=(c == 0), stop=(c == DC - 1))
        for ab in range(DC):
            nc.vector.tensor_copy(mcsb[:, ab, 0:HD], mpts[ab][:])

        # Mc exchange: spill own columns, token AllReduce, read peer's
        nc.sync.dma_start(
            Msh[bass.ds(svo, 1), :, :].rearrange("s (c p) n -> p (s c) n",
                                                 p=P),
            mcsb[:, :, 0:HD])
        tktm = stage.tile([1, 2], F32, tag="tkt", name="tktm")
        for e in range(2):
            nc.sync.dma_start(
                tktm[0:1, e:e + 1],
                Msh[bass.ds(svo, 1), e:e + 1, 0:1].rearrange(
                    "s c n -> c s n").bitcast(F32))
        nc.sync.dma_start(tokm[:], tktm[:])
        mbar = nc.gpsimd.collective_compute(
            "AllReduce", mybir.AluOpType.add, replica_groups=PAIRS,
            ins=[tokm], outs=[tokmo])
        mld = nc.scalar.dma_start(
            mcsb[:, :, HD:D],
            Msh[bass.ds(svp, 1), :, :].rearrange("s (c p) n -> p (s c) n",
                                                 p=P))
        add_dep_helper(mld.ins, mbar.ins, reason="peer Mc after pair barrier")

        # ------------- Phase B2: out = x_own @ [Mc_own | Mc_peer] ---------
        # dh=0 is the own column half (computed first, hiding the exchange);
        # the dynamic slot offset maps it back to the global column half
        for dh in range(2):
            sl = svo if dh == 0 else svp
            for lb in range(LH // P):
                pt = psum.tile([P, 512], F32, tag="ps", name=f"o_{dh}_{lb}")
                for c in range(DC):
                    nc.tensor.matmul(
                        pt[:], xthsb[:, c, lb * P:(lb + 1) * P],
                        mcsb[:, c, dh * HD:(dh + 1) * HD],
                        start=(c == 0), stop=(c == DC - 1))
                ot = opool.tile([P, 1, HD], F32, tag="ot",
                                name=f"ot_{dh}_{lb}")
                nc.vector.tensor_copy(ot[:, 0], pt[:])
                nc.sync.dma_start(
                    out[bass.ds(sl, 1), lb * P:(lb + 1) * P, :].rearrange(
                        "s p n -> p s n"), ot[:])

    nc.compile()
    return nc


_NC_CACHE = {}


def _get_nc():
    if "nc" not in _NC_CACHE:
        _NC_CACHE["nc"] = build_nc()
    return _NC_CACHE["nc"]


def run(inputs, trace=False):
    """Run the kernel on all 8 cores. Returns (full_output, BassKernelResults)."""
    x = np.asarray(inputs["x"], dtype=np.float32)
    Wq = np.asarray(inputs["Wq"], dtype=np.float32)
    Wk = np.asarray(inputs["Wk"], dtype=np.float32)
    Wv = np.asarray(inputs["Wv"], dtype=np.float32)

    inv_sqrt_d = np.float32(1.0 / np.sqrt(D))
    wq_s = np.ascontiguousarray(Wq * inv_sqrt_d)  # natural [e, a] layout
    wkT = np.ascontiguousarray(Wk.T)
    wvT = np.ascontiguousarray(Wv.T)

    in_maps = []
    for c in range(N_CORES):
        b, h = c // 2, c % 2
        in_maps.append({
            "xn": np.ascontiguousarray(x[b, h * LH:(h + 1) * LH, :]),
            "xth": np.ascontiguousarray(x[b].T[:, h * LH:(h + 1) * LH]),
            "wq": wq_s, "wkT": wkT,
            "wvT": np.ascontiguousarray(wvT[:, h * HD:(h + 1) * HD]),
            "slots": np.array([[h, 1 - h]], dtype=np.uint32),
        })

    nc = _get_nc()
    res = run_bass_kernel_spmd(nc, in_maps, list(range(N_CORES)), trace=trace)

    full = np.empty((B, L, D), dtype=np.float32)
    for c in range(N_CORES):
        b, h = c // 2, c % 2
        oc = res.results[c]["out"]  # [2, LH, HD]; dim 0 = global col half
        full[b, h * LH:(h + 1) * LH, :] = (
            oc.transpose(1, 0, 2).reshape(LH, D))
    return full, res


def kernel(**inputs):
    full, _ = run(inputs, trace=False)
    return full
